# revision 1
# baseline (speedup 1.0000x reference)
"""Trainium2 Bass kernel for nn_Cube_Norm (segment min/max normalize).

Reference semantics (per graph g of 256 nodes, per dim d):
    tmax = max_n x[g,n,d]; tmin = min_n x[g,n,d]
    mid = (tmax+tmin)/2; ldv = max((tmax-tmin)/2, 1e-12)
    out[g,n,d] = (x[g,n,d] - mid) / ldv

Sharding: 1024 graphs -> 8 cores x 128 graphs (row-sharded at graph
boundaries). Per core, 4 rounds of 32 graphs; each graph occupies 4 SBUF
partitions (64 nodes each): every round is a [128, 19200] fp32 tile with
contiguous DMA in/out (exactly 2x HBM traffic), double-buffered.

Engine split (HW-probed on this silicon):
  - A DVE op with two SBUF operands (or an sb->sb copy) crawls 4-10x
    while GpSimd is busy: they arbitrate a shared SBUF port pair and the
    loser blocks for the whole instruction. A DVE op with one SBUF
    stream + PSUM for the rest runs at FULL speed alongside GpSimd.
  - So: DVE folds are PSUM-accumulator chains TT(chunk_sb, acc_pm ->
    acc_pm); the cross-partition stat tree + math run on PSUM scratch
    (with small ACT sb<->psum copies; ACT has its own ports); the
    DVE normalize slice reads stats from PSUM. GpSimd runs the bulk of
    the normalize from SBUF concurrently; in round 0 (no normalize yet)
    it folds a share of the chunks instead.
  - All stat math is TT/reciprocal with broadcast const tiles - never
    tensor_scalar/copy on DVE (2-port modes would grab the shared pair).
  - Loads ride the sync HWDGE ring; stores and stat DMAs ride the
    scalar ring, so stores never head-of-line-block loads.
"""

import numpy as np

NUM_GRAPHS = 1024
NPG = 256            # nodes per graph
D = 300              # embed dim
N_CORES = 8
GPC = NUM_GRAPHS // N_CORES   # 128 graphs per core
ROWS_PER_CORE = GPC * NPG     # 32768
P = 128              # SBUF partitions
Q = 4                # partitions per graph
NPP = NPG // Q       # 64 nodes per partition
GPR = P // Q         # 32 graphs per round
ROUNDS = GPC // GPR  # 4
FREE = NPP * D       # 19200 fp32 per partition per round
ROWS_PER_ROUND = GPR * NPG    # 8192
EPS = 1e-12

CH = 1200            # fold chunk (PSUM acc width; 300*2^k)
NCH = FREE // CH     # 16 chunks

# normalize node-split: DVE handles ND_* nodes of 64, GpSimd the rest
ND_MID = 12
ND_LAST = 43

_CACHE = {}


def _split_multi_waits(nc, mybir, max_waits=1):
    """walrus in this container rejects >N sync waits on one instruction;
    hoist extras into standalone NOPs on the same engine just before."""
    n = 0
    for f in nc.m.functions:
        for bb in f.blocks:
            new_insts = []
            for inst in bb.instructions:
                si = inst.sync_info
                if si is not None and si.on_wait and len(si.on_wait) > max_waits:
                    extra = list(si.on_wait[: len(si.on_wait) - max_waits])
                    keep = list(si.on_wait[len(si.on_wait) - max_waits:])
                    for j, w in enumerate(extra):
                        new_insts.append(
                            mybir.InstNoOp(
                                name=f"{inst.name}-sw{j}",
                                sync_info=mybir.SyncInfo(on_wait=[w], on_update=[]),
                                bass_nofuse=True,
                                engine=inst.engine,
                            )
                        )
                        n += 1
                    inst.sync_info = mybir.SyncInfo(
                        on_wait=keep, on_update=list(si.on_update)
                    )
                new_insts.append(inst)
            bb.instructions.clear()
            for i in new_insts:
                bb.add_instruction(i)
    return n


def _build():
    import concourse.bass as bass
    import concourse.tile as tile
    from concourse import mybir

    F32 = mybir.dt.float32
    OP = mybir.AluOpType

    nc = bass.Bass()
    x = nc.dram_tensor("x", [ROWS_PER_CORE, D], F32, kind="ExternalInput")
    y = nc.dram_tensor("y", [ROWS_PER_CORE, D], F32, kind="ExternalOutput")

    with tile.TileContext(nc) as tc:
        with tc.tile_pool(name="data", bufs=2) as data_pool, \
             tc.tile_pool(name="rep", bufs=2) as rep_pool, \
             tc.tile_pool(name="sml", bufs=1) as sml_pool, \
             tc.tile_pool(name="acc", bufs=2, space="PSUM") as acc_pool, \
             tc.tile_pool(name="prep", bufs=1, space="PSUM") as prep_pool:
            # broadcast consts for stat math ([GPR,1], read via rd0)
            cst = sml_pool.tile([GPR, 4], F32, tag="cst")
            nc.vector.memset(cst[:, 0:1], 0.5)
            nc.vector.memset(cst[:, 1:2], -0.5)
            nc.vector.memset(cst[:, 2:3], EPS)
            half_b = cst[:, 0:1].broadcast_to([GPR, D])
            neghalf_b = cst[:, 1:2].broadcast_to([GPR, D])
            eps_b = cst[:, 2:3].broadcast_to([GPR, D])

            # persistent PSUM stats [P, mid|rinv] (single buffer: its
            # reader (norm r-1) and writer (stats r) are both in-order
            # DVE ops, so WAR needs no extra buffering)
            pm_rep = prep_pool.tile([P, 2 * D], F32, tag="pmrep")

            live = {}  # r -> (t, rep_sb, parity) awaiting normalize+store
            for r in range(ROUNDS + 1):
                if r < ROUNDS:
                    rows = slice(r * ROWS_PER_ROUND, (r + 1) * ROWS_PER_ROUND)

                    # load in four quarters so folds start as data streams in
                    t = data_pool.tile([P, FREE], F32, tag="t")
                    xr = x[rows, :].rearrange("(p f) d -> p (f d)", p=P)
                    FQ = FREE // 4
                    for qd in range(4):
                        nc.sync.dma_start(
                            t[:, qd * FQ:(qd + 1) * FQ], xr[:, qd * FQ:(qd + 1) * FQ]
                        )

                if r >= 1:
                    # normalize round r-1: out = (x - mid) * rinv, in place.
                    # DVE slices read stats from PSUM (shared-pair-free) and
                    # store via the scalar ring (they complete early, so they
                    # can't block gather/replicate behind them). GpSimd slices
                    # read stats from SBUF and store via their own SWDGE ring
                    # right after their compute - keeping late-completing
                    # stores off both HWDGE rings entirely.
                    tp, rep_sb = live.pop(r - 1)
                    rowsp = slice((r - 1) * ROWS_PER_ROUND, r * ROWS_PER_ROUND)
                    tv3 = tp[:].rearrange("p (n d) -> p n d", n=NPP, d=D)
                    yr = y[rowsp, :].rearrange("(p f) d -> p (f d)", p=P)

                    nd = ND_LAST if r == ROUNDS else ND_MID
                    ng1 = (NPP - nd + 1) // 2
                    if r == ROUNDS:
                        # drain round: chunk the DVE slice so stores overlap
                        # the remaining compute instead of a serial tail
                        dsegs = [(a, min(a + 15, nd)) for a in range(0, nd, 15)]
                    else:
                        dsegs = [(0, nd)]
                    segs = [(a, b, nc.vector) for a, b in dsegs] + [
                        (nd, nd + ng1, nc.gpsimd),
                        (nd + ng1, NPP, nc.gpsimd),
                    ]
                    for n0, n1, eng in segs:
                        ns = slice(n0, n1)
                        H = n1 - n0
                        if eng is nc.vector:
                            mid_b = pm_rep[:, 0:D] \
                                .unsqueeze(1).broadcast_to([P, H, D])
                            rinv_b = pm_rep[:, D:2 * D] \
                                .unsqueeze(1).broadcast_to([P, H, D])
                        else:
                            mid_b = rep_sb[:, 0:D] \
                                .unsqueeze(1).broadcast_to([P, H, D])
                            rinv_b = rep_sb[:, D:2 * D] \
                                .unsqueeze(1).broadcast_to([P, H, D])
                        eng.tensor_tensor(
                            tv3[:, ns, :], tv3[:, ns, :], mid_b, op=OP.subtract
                        )
                        eng.tensor_tensor(
                            tv3[:, ns, :], tv3[:, ns, :], rinv_b, op=OP.mult
                        )
                        if eng is nc.vector:
                            nc.scalar.dma_start(
                                yr[:, n0 * D:n1 * D], tp[:, n0 * D:n1 * D]
                            )
                        else:
                            nc.gpsimd.dma_start(
                                yr[:, n0 * D:n1 * D], tp[:, n0 * D:n1 * D]
                            )

                if r < ROUNDS:
                    # per-partition partials: s cols [0:D]=max, [D:2D]=min.
                    # DVE chain keeps the accumulator in PSUM (in0 = sbuf
                    # chunk via the dedicated read port, in1/out = PSUM), so
                    # it never touches the DVE/GpSimd shared SBUF ports.
                    # ACT fully blocks whenever GpSimd is busy (and GpSimd
                    # runs back-to-back rounds), so ACT is not used at all in
                    # steady state: the acc inits are DVE copies too (sbuf
                    # read + PSUM write never touches the shared pair).
                    # (GpSimd can't help fold: Pool TT rejects max/min ops.)
                    s = sml_pool.tile([P, 2 * D], F32, tag="s")
                    accs = {}
                    for si in (0, 1):
                        accs[si] = acc_pool.tile(
                            [P, CH], F32, tag="acc", name=f"acc{r}_{si}"
                        )
                        nc.vector.tensor_copy(accs[si][:], t[:, 0:CH])
                    for si, op in ((0, OP.max), (1, OP.min)):
                        acc = accs[si]
                        for c in range(1, NCH):
                            nc.vector.tensor_tensor(
                                acc[:], t[:, c * CH:(c + 1) * CH], acc[:], op=op
                            )
                        h = sml_pool.tile([P, CH // 2], F32, tag="h")
                        m = CH // 2
                        while m > D:
                            nc.vector.tensor_copy(h[:, 0:m], acc[:, m:2 * m])
                            nc.vector.tensor_tensor(
                                acc[:, 0:m], h[:, 0:m], acc[:, 0:m], op=op
                            )
                            m //= 2
                        nc.vector.tensor_copy(h[:, 0:D], acc[:, D:2 * D])
                        nc.vector.tensor_tensor(
                            s[:, si * D:(si + 1) * D], h[:, 0:D],
                            acc[:, 0:D], op=op,
                        )

                    # gather the 4 partials of each graph onto one partition.
                    # Scalar ring: the sync ring is reserved for loads so the
                    # next round's loads are never head-of-line-blocked; the
                    # scalar ring carries only early-completing work (acc
                    # inits, the DVE-slice store, gather, replicate).
                    tq = sml_pool.tile([GPR, Q, 2 * D], F32, tag="tq")
                    for q in range(Q):
                        nc.scalar.dma_start(tq[:, q, :], s[q::Q, :])

                if r < ROUNDS:
                    # cross-partition fold tree + stat math on PSUM scratch
                    # (in0 always a single SBUF stream, in1/out PSUM). Borrows
                    # the fold-acc pool slot (same 9600 B/partition; the fold
                    # chains of this round are done with it by now) - only
                    # pages [0:2] of the [GPR, 4, 2D] view are used.
                    scr = acc_pool.tile([GPR, 2, 2 * D], F32, tag="acc")
                    nc.vector.tensor_copy(scr[:], tq[:, 2:4, :])
                    nc.vector.tensor_tensor(
                        scr[:, :, 0:D], tq[:, 0:2, 0:D], scr[:, :, 0:D],
                        op=OP.max,
                    )
                    nc.vector.tensor_tensor(
                        scr[:, :, D:2 * D], tq[:, 0:2, D:2 * D],
                        scr[:, :, D:2 * D], op=OP.min,
                    )
                    h2 = sml_pool.tile([GPR, 2 * D], F32, tag="h2")
                    nc.vector.tensor_copy(h2[:], scr[:, 1, :])
                    nc.vector.tensor_tensor(
                        scr[:, 0, 0:D], h2[:, 0:D], scr[:, 0, 0:D], op=OP.max
                    )
                    nc.vector.tensor_tensor(
                        scr[:, 0, D:2 * D], h2[:, D:2 * D], scr[:, 0, D:2 * D],
                        op=OP.min,
                    )
                    # pmax = scr[:,0,0:D], pmin = scr[:,0,D:2D] (PSUM)
                    pmin_sb = sml_pool.tile([GPR, D], F32, tag="pminsb")
                    nc.vector.tensor_copy(pmin_sb[:], scr[:, 0, D:2 * D])
                    # mid = (pmax+pmin)*0.5 -> scr[:,1,0:D]
                    nc.vector.tensor_tensor(
                        scr[:, 1, 0:D], pmin_sb[:], scr[:, 0, 0:D], op=OP.add
                    )
                    nc.vector.tensor_tensor(
                        scr[:, 1, 0:D], half_b, scr[:, 1, 0:D], op=OP.mult
                    )
                    # rinv = 1/max((pmin-pmax)*-0.5, EPS) -> scr[:,1,D:2D]
                    nc.vector.tensor_tensor(
                        scr[:, 1, D:2 * D], pmin_sb[:], scr[:, 0, 0:D],
                        op=OP.subtract,
                    )
                    nc.vector.tensor_tensor(
                        scr[:, 1, D:2 * D], neghalf_b, scr[:, 1, D:2 * D],
                        op=OP.mult,
                    )
                    nc.vector.tensor_tensor(
                        scr[:, 1, D:2 * D], eps_b, scr[:, 1, D:2 * D], op=OP.max
                    )
                    nc.vector.reciprocal(scr[:, 0, 0:D], scr[:, 1, D:2 * D])
                    # ab_sb = (mid, rinv) on 32 partitions
                    ab = sml_pool.tile([GPR, 2 * D], F32, tag="ab")
                    nc.vector.tensor_copy(ab[:, 0:D], scr[:, 1, 0:D])
                    nc.vector.tensor_copy(ab[:, D:2 * D], scr[:, 0, 0:D])

                    # replicate stats to all Q partitions of each graph
                    # (scalar ring; only early-completing work lives there)
                    rep_sb = rep_pool.tile([P, 2 * D], F32, tag="repsb")
                    for q in range(Q):
                        nc.scalar.dma_start(rep_sb[q::Q, :], ab[:, :])
                    # and into PSUM for the DVE slice (DVE copy: ACT
                    # would block under GpSimd)
                    nc.vector.tensor_copy(pm_rep[:], rep_sb[:])

                    live[r] = (t, rep_sb)

    _split_multi_waits(nc, mybir)
    return nc


def kernel(tensor, batch_list=None, **_ignored):
    """Full-input entry point: tensor [262144, 300] fp32 -> [262144, 300] fp32.

    batch_list is the constant 256-per-graph layout baked into this kernel.
    """
    from concourse.bass_utils import run_bass_kernel_spmd

    tensor = np.ascontiguousarray(np.asarray(tensor), dtype=np.float32)
    assert tensor.shape == (NUM_GRAPHS * NPG, D), tensor.shape

    if "nc" not in _CACHE:
        _CACHE["nc"] = _build()
    nc = _CACHE["nc"]

    in_maps = [
        {"x": tensor[c * ROWS_PER_CORE:(c + 1) * ROWS_PER_CORE]}
        for c in range(N_CORES)
    ]
    res = run_bass_kernel_spmd(nc, in_maps, core_ids=list(range(N_CORES)))
    out = np.concatenate([res.results[c]["y"] for c in range(N_CORES)], axis=0)
    return out



# revision 3
# speedup vs baseline: 1.1358x; 1.1358x over previous
"""Trainium2 Bass kernel for nn_Cube_Norm (segment min/max normalize), v2.

Reference semantics (per graph g of 256 nodes, per dim d):
    tmax = max_n x[g,n,d]; tmin = min_n x[g,n,d]
    mid = (tmax+tmin)/2; ldv = max((tmax-tmin)/2, 1e-12)
    out[g,n,d] = (x[g,n,d] - mid) / ldv

Sharding: 1024 graphs -> 8 cores x 128 graphs. Per core, 4 rounds of 32
graphs; each graph occupies 4 SBUF partitions (64 nodes each): every round
is a [128, 19200] fp32 tile with contiguous DMA in/out, double-buffered.

v2 changes vs baseline (HW-probed on this silicon):
  - ALL copies (fold-acc inits, tree halving copies, stat copies, psum
    stat replication) moved to the otherwise-idle ACT engine (~1 elem/cyc,
    own SBUF/PSUM ports, runs concurrently with both DVE and Pool).
  - Normalize rebalanced: DVE takes ND nodes/round (TT sub+mult at 1x,
    stats broadcast from PSUM so no shared-port grab), Pool the rest
    (stats from SBUF). Pool is the cheaper-per-op engine for nothing --
    it runs ~0.46 elem/cyc -- but it has no fold duty, so it gets the
    bulk. Pool intentionally runs hotter than the DMA window; the
    double-buffered t tile gives a full round of lag tolerance.
  - Tail round (no following fold) flips the split: DVE takes NDT=44
    nodes, halving the serial epilogue.
  - All stores ride the scalar HWDGE ring, issued from the ACT queue:
    the Pool instruction queue carries ONLY TT compute ops.
  - Engine-port discipline (probed): every DVE op streams at most ONE
    SBUF tensor (other operands PSUM); 2-port DVE modes (tensor_scalar,
    CAST, 2-SBUF-operand TT) are never used while Pool is busy - the
    shared SBUF port pair is an exclusive per-instruction lock and the
    loser fully blocks.
"""

import numpy as np

NUM_GRAPHS = 1024
NPG = 256            # nodes per graph
D = 300              # embed dim
N_CORES = 8
GPC = NUM_GRAPHS // N_CORES   # 128 graphs per core
ROWS_PER_CORE = GPC * NPG     # 32768
P = 128              # SBUF partitions
Q = 4                # partitions per graph
NPP = NPG // Q       # 64 nodes per partition
GPR = P // Q         # 32 graphs per round
ROUNDS = GPC // GPR  # 4
FREE = NPP * D       # 19200 fp32 per partition per round
ROWS_PER_ROUND = GPR * NPG    # 8192
EPS = 1e-12

CH = 1200            # fold chunk (PSUM acc width; 300*2^k)
NCH = FREE // CH     # 16 chunks

ND = 16              # nodes (of 64) normalized by DVE in steady rounds
NDT = 44             # nodes normalized by DVE in the tail round

_CACHE = {}


def _split_multi_waits(nc, mybir, max_waits=1):
    """walrus in this container rejects >N sync waits on one instruction;
    hoist extras into standalone NOPs on the same engine just before."""
    n = 0
    for f in nc.m.functions:
        for bb in f.blocks:
            new_insts = []
            for inst in bb.instructions:
                si = inst.sync_info
                if si is not None and si.on_wait and len(si.on_wait) > max_waits:
                    extra = list(si.on_wait[: len(si.on_wait) - max_waits])
                    keep = list(si.on_wait[len(si.on_wait) - max_waits:])
                    for j, w in enumerate(extra):
                        new_insts.append(
                            mybir.InstNoOp(
                                name=f"{inst.name}-sw{j}",
                                sync_info=mybir.SyncInfo(on_wait=[w], on_update=[]),
                                bass_nofuse=True,
                                engine=inst.engine,
                            )
                        )
                        n += 1
                    inst.sync_info = mybir.SyncInfo(
                        on_wait=keep, on_update=list(si.on_update)
                    )
                new_insts.append(inst)
            bb.instructions.clear()
            for i in new_insts:
                bb.add_instruction(i)
    return n


def _build():
    import concourse.bass as bass
    import concourse.tile as tile
    from concourse import mybir

    F32 = mybir.dt.float32
    BF16 = mybir.dt.bfloat16
    OP = mybir.AluOpType

    nc = bass.Bass()
    x = nc.dram_tensor("x", [ROWS_PER_CORE, D], F32, kind="ExternalInput")
    y = nc.dram_tensor("y", [ROWS_PER_CORE, D], F32, kind="ExternalOutput")

    with tile.TileContext(nc) as tc:
        with tc.tile_pool(name="data", bufs=2) as data_pool, \
             tc.tile_pool(name="rep", bufs=2) as rep_pool, \
             tc.tile_pool(name="sml", bufs=1) as sml_pool, \
             tc.tile_pool(name="acc", bufs=2, space="PSUM") as acc_pool, \
             tc.tile_pool(name="pm", bufs=1, space="PSUM") as pm_pool:
            # broadcast consts for stat math ([P,1], read via rd0)
            cst = sml_pool.tile([P, 4], F32, tag="cst")
            nc.vector.memset(cst[:, 0:1], 0.5)
            nc.vector.memset(cst[:, 1:2], -0.5)
            nc.vector.memset(cst[:, 2:3], EPS)
            half_b = cst[:, 0:1].broadcast_to([P, D])
            neghalf_b = cst[:, 1:2].broadcast_to([P, D])
            eps_b = cst[:, 2:3].broadcast_to([P, D])

            # tree scratch (SBUF): [0:600]=max half, [600:1200]=min half
            h = sml_pool.tile([P, CH], F32, tag="h")
            s = sml_pool.tile([P, 2 * D], F32, tag="s")
            # bf16 normalize output tiles for the Pool quarters (1..3):
            # sub writes here instead of in-place, so the t quarter is
            # freed for the next-next round's load as soon as the SUB has
            # read it -- not only after mult+store.  bf16 halves Pool's
            # mult cost and the store casts back to fp32 in the SWDGE DMA.
            uA = sml_pool.tile([P, FREE // 4], BF16, tag="uA")
            uB = sml_pool.tile([P, FREE // 4], BF16, tag="uB")
            uC = sml_pool.tile([P, FREE // 4], BF16, tag="uC")
            uqs = [uA, uB, uC]
            u0 = None  # ND==16: quarter 0 is fully DVE-normalized
            pmin_sb = sml_pool.tile([P, D], F32, tag="pminsb")

            live = {}
            FQ = FREE // 4          # 4800 elems = 16 nodes per quarter tile
            NQ = NPP // 4           # 16 nodes per quarter
            for r in range(ROUNDS + 1):
                pending_stores = []
                if r < ROUNDS:
                    rows = slice(r * ROWS_PER_ROUND, (r + 1) * ROWS_PER_ROUND)
                    # t split into 4 quarter tiles so the next-next round's
                    # load of a quarter only WAR-waits on THAT quarter's
                    # normalize (not on Pool finishing the whole tile).
                    tqs = [data_pool.tile([P, FQ], F32, tag=f"t{qd}",
                                          name=f"t{qd}_{r}")
                           for qd in range(4)]
                    xr = x[rows, :].rearrange("(p f) d -> p (f d)", p=P)
                    for qd in range(4):
                        nc.sync.dma_start(
                            tqs[qd][:], xr[:, qd * FQ:(qd + 1) * FQ]
                        )

                if r >= 1:
                    # ---- normalize round r-1: out = (x - mid) * rinv, in
                    # place in t, fp32 throughout. DVE slices read stats
                    # from PSUM (pm); Pool slices from SBUF (rep). Stores
                    # ride the scalar HWDGE ring (ACT queue).
                    tps, rep, pm, rbrep = live.pop(r - 1)
                    rowsp = slice((r - 1) * ROWS_PER_ROUND, r * ROWS_PER_ROUND)
                    yr = y[rowsp, :].rearrange("(p f) d -> p (f d)", p=P)

                    nd = NDT if r == ROUNDS else ND
                    # (quarter, local n0, local n1, engine); nd nodes on DVE
                    # in 8-node slices, the rest on Pool in 16-node slices
                    segs = []
                    for a in range(0, nd, 8):
                        b = min(a + 8, nd)
                        q0, n0l = divmod(a, NQ)
                        q1 = (b - 1) // NQ
                        assert q0 == q1, (a, b)
                        segs.append((q0, n0l, n0l + (b - a), nc.vector))
                    a = nd
                    while a < NPP:
                        q0, n0l = divmod(a, NQ)
                        b = min((q0 + 1) * NQ, NPP)
                        segs.append((q0, n0l, NQ, nc.gpsimd))
                        a = b
                    for qd, n0, n1, eng in segs:
                        tp = tps[qd]
                        tv3 = tp[:].rearrange("p (n d) -> p n d", n=NQ, d=D)
                        ns = slice(n0, n1)
                        H = n1 - n0
                        g0, g1 = qd * NQ + n0, qd * NQ + n1
                        if eng is nc.vector:
                            # fp32 in place in t; stats from PSUM so no
                            # shared-port grab; store on the scalar ring
                            # (issues early, can't block the ACT queue).
                            mid_b = pm[:, 0:D].unsqueeze(1).broadcast_to([P, H, D])
                            rinv_b = pm[:, D:2 * D].unsqueeze(1).broadcast_to([P, H, D])
                            eng.tensor_tensor(
                                tv3[:, ns, :], tv3[:, ns, :], mid_b, op=OP.subtract
                            )
                            eng.tensor_tensor(
                                tv3[:, ns, :], tv3[:, ns, :], rinv_b, op=OP.mult
                            )
                            # store issue deferred to end of block: the bulk
                            # store must not sit on the scalar ring ahead of
                            # the fold's tiny gather/replicate DMAs (FIFO
                            # per ring). It has a full round of slack before
                            # this t quarter is reloaded.
                            pending_stores.append(
                                (yr[:, g0 * D:g1 * D], tp[:, n0 * D:n1 * D])
                            )
                        else:
                            # Pool: sub t -> u (bf16), freeing the t quarter
                            # at sub time; bf16 mult; SWDGE cast store
                            # (self-issued: zero queue wait).
                            if qd == 0:
                                up, uw = u0, 16 - ND
                                un0, un1 = n0 - ND, n1 - ND
                            else:
                                up, uw = uqs[qd - 1], NQ
                                un0, un1 = n0, n1
                            uv3 = up[:].rearrange(
                                "p (n d) -> p n d", n=uw, d=D)
                            uns = slice(un0, un1)
                            mid_b = rep[:, 0:D].unsqueeze(1).broadcast_to([P, H, D])
                            rb_b = rbrep[:].unsqueeze(1).broadcast_to([P, H, D])
                            eng.tensor_tensor(
                                uv3[:, uns, :], tv3[:, ns, :], mid_b, op=OP.subtract
                            )
                            eng.tensor_tensor(
                                uv3[:, uns, :], uv3[:, uns, :], rb_b, op=OP.mult
                            )
                            nc.gpsimd.dma_start(
                                yr[:, g0 * D:g1 * D], up[:, un0 * D:un1 * D]
                            )

                if r < ROUNDS:
                    # ---- fold round r: per-partition max/min partials.
                    # ACT does the acc inits; DVE runs the two chains
                    # interleaved (each op: 1 SBUF stream in0, acc in PSUM).
                    am = acc_pool.tile([P, CH], F32, tag="acc", name=f"am{r}")
                    an = acc_pool.tile([P, CH], F32, tag="acc", name=f"an{r}")
                    nc.scalar.copy(am[:], tqs[0][:, 0:CH])
                    nc.scalar.copy(an[:], tqs[0][:, 0:CH])
                    for c in range(1, NCH):
                        ch = tqs[c // 4][:, (c % 4) * CH:(c % 4 + 1) * CH]
                        nc.vector.tensor_tensor(am[:], ch, am[:], op=OP.max)
                        nc.vector.tensor_tensor(an[:], ch, an[:], op=OP.min)
                    # tree: 1200 -> 600 -> 300, copies on ACT
                    nc.scalar.copy(h[:, 0:600], am[:, 600:1200])
                    nc.scalar.copy(h[:, 600:1200], an[:, 600:1200])
                    nc.vector.tensor_tensor(
                        am[:, 0:600], h[:, 0:600], am[:, 0:600], op=OP.max)
                    nc.vector.tensor_tensor(
                        an[:, 0:600], h[:, 600:1200], an[:, 0:600], op=OP.min)
                    nc.scalar.copy(h[:, 0:300], am[:, 300:600])
                    nc.scalar.copy(h[:, 600:900], an[:, 300:600])
                    nc.vector.tensor_tensor(
                        s[:, 0:D], h[:, 0:300], am[:, 0:300], op=OP.max)
                    nc.vector.tensor_tensor(
                        s[:, D:2 * D], h[:, 600:900], an[:, 0:300], op=OP.min)

                    # butterfly fold across each graph's 4 partitions via
                    # stream_shuffle (XOR-lane exchange): two steps and every
                    # partition holds its graph's full pmax|pmin. Replaces
                    # the gather DMAs + 32-partition cross tree + replicate
                    # DMAs (~25us of serial small-DMA latency per round).
                    M1 = [i ^ 1 for i in range(32)]
                    M2 = [i ^ 2 for i in range(32)]
                    scr = acc_pool.tile([P, 2 * D], F32, tag="acc",
                                        name=f"scr{r}")
                    sh = sml_pool.tile([P, 2 * D], F32, tag="sh",
                                       name=f"sh{r}")
                    sf = sml_pool.tile([P, 2 * D], F32, tag="sf",
                                       name=f"sf{r}")
                    nc.scalar.copy(scr[:], s[:])
                    nc.vector.stream_shuffle(sh[:], s[:], M1)
                    nc.vector.tensor_tensor(
                        scr[:, 0:D], sh[:, 0:D], scr[:, 0:D], op=OP.max)
                    nc.vector.tensor_tensor(
                        scr[:, D:2 * D], sh[:, D:2 * D], scr[:, D:2 * D],
                        op=OP.min)
                    nc.scalar.copy(sf[:], scr[:])
                    nc.vector.stream_shuffle(sh[:], sf[:], M2)
                    nc.vector.tensor_tensor(
                        scr[:, 0:D], sh[:, 0:D], scr[:, 0:D], op=OP.max)
                    nc.vector.tensor_tensor(
                        scr[:, D:2 * D], sh[:, D:2 * D], scr[:, D:2 * D],
                        op=OP.min)
                    # pmax = scr[:,0:D], pmin = scr[:,D:2D] on ALL partitions
                    nc.scalar.copy(pmin_sb[:], scr[:, D:2 * D])
                    pm = pm_pool.tile([P, 2 * D], F32, tag="pm")
                    # mid = (pmax+pmin)*0.5 -> pm[:,0:D]
                    nc.vector.tensor_tensor(
                        pm[:, 0:D], pmin_sb[:], scr[:, 0:D], op=OP.add)
                    nc.vector.tensor_tensor(
                        pm[:, 0:D], half_b, pm[:, 0:D], op=OP.mult)
                    # ldv = max((pmin-pmax)*-0.5, EPS) -> pm[:,D:2D]
                    nc.vector.tensor_tensor(
                        pm[:, D:2 * D], pmin_sb[:], scr[:, 0:D], op=OP.subtract)
                    nc.vector.tensor_tensor(
                        pm[:, D:2 * D], neghalf_b, pm[:, D:2 * D], op=OP.mult)
                    nc.vector.tensor_tensor(
                        pm[:, D:2 * D], eps_b, pm[:, D:2 * D], op=OP.max)
                    nc.vector.reciprocal(scr[:, 0:D], pm[:, D:2 * D])
                    nc.vector.tensor_copy(pm[:, D:2 * D], scr[:, 0:D])
                    # SBUF copies for the Pool slices: rep fp32, rbrep bf16
                    rep = rep_pool.tile([P, 2 * D], F32, tag="rep")
                    rbrep = rep_pool.tile([P, D], BF16, tag="rbrep")
                    nc.scalar.copy(rep[:], pm[:])
                    nc.scalar.copy(rbrep[:], scr[:, 0:D])

                    live[r] = (tqs, rep, pm, rbrep)

                for dst, src in pending_stores:
                    nc.scalar.dma_start(dst, src)

    _split_multi_waits(nc, mybir)
    return nc


def kernel(tensor, batch_list=None, **_ignored):
    """Full-input entry point: tensor [262144, 300] fp32 -> same-shape fp32.

    batch_list is the constant 256-per-graph layout baked into this kernel.
    """
    from concourse.bass_utils import run_bass_kernel_spmd

    tensor = np.ascontiguousarray(np.asarray(tensor), dtype=np.float32)
    assert tensor.shape == (NUM_GRAPHS * NPG, D), tensor.shape

    if "nc" not in _CACHE:
        _CACHE["nc"] = _build()
    nc = _CACHE["nc"]

    in_maps = [
        {"x": tensor[c * ROWS_PER_CORE:(c + 1) * ROWS_PER_CORE]}
        for c in range(N_CORES)
    ]
    res = run_bass_kernel_spmd(nc, in_maps, core_ids=list(range(N_CORES)))
    out = np.concatenate([res.results[c]["y"] for c in range(N_CORES)], axis=0)
    return out
